# revision 25
# baseline (speedup 1.0000x reference)
"""Trainium2 Bass kernel for a 2-layer "BiGRU" (batch-flipped, per reference).

Structure exploited:
  * The reference's "backward" direction flips the BATCH dim, not time. In
    flipped coordinates (track hb_hat[b] := hb[B-1-b]) every GRU cell
    consumes the UNFLIPPED input stream; the flip appears only when
    concatenating layer-0 outputs into layer-1's input and when emitting
    the final output (done on host).
  * Batch 64 is sharded over 8 cores in flip-closed groups of 8
    (globals [4d..4d+3, 63-4d-3..63-4d]), so the flip is a local reversal
    and each core is fully independent (no collectives).
  * Per step, the recurrent matmul streams whh through the PE array with
    h^T as the stationary operand (M=8 columns). The four cells sit in
    different PE column groups (output partitions 0/32/64/96 bases), so
    their weight streams run concurrently.

Self-contained: hardcodes all shapes from the problem spec.
"""

import numpy as np

from concourse import bass, bacc, tile
from concourse.bass import mybir
from concourse.bass_utils import run_bass_kernel_spmd
from concourse.masks import make_identity

SEQ, BATCH, IN, HID = 512, 64, 512, 512
G3 = 3 * HID  # 1536
BC = 8  # local batch per core
NCORES = 8
FP32 = mybir.dt.float32
BF16 = mybir.dt.bfloat16

# cells: (name, psum base partition, n input K-chunks of 128)
L0_CELLS = [("f0", 0, 4), ("b0", 32, 4)]
L1_CELLS = [("f1", 0, 8), ("b1", 32, 8)]
ALL_CELLS = L0_CELLS + L1_CELLS


def _blob_layout(S):
    """Free-dim offsets (in bf16 elements) inside the single load blob."""
    off = {}
    cur = 0
    for k in range(4):
        off[f"xT{k}"] = cur
        cur += S * BC
    for cname, _, kx in ALL_CELLS:
        for k in range(kx):
            off[f"w_{cname}{k}"] = cur
            cur += G3
        for k in range(4):
            off[f"u_{cname}{k}"] = cur
            cur += G3
    off["bias"] = cur          # (partition row c = cell c, 2048 wide)
    cur += 2048
    off["onehot"] = cur        # (4, 4*BC): cell c's one-hot rows at cols c*BC
    cur += 4 * BC
    return off, cur


def build_core_program(S):
    nc = bacc.Bacc(None, target_bir_lowering=False)

    # All bf16 payloads are host-packed into fp32 words (pairs of bf16 per
    # word), shipped as ONE blob, and bitcast on device. A single DMA means
    # every consumer waits on at most one DMA-queue semaphore (this walrus
    # rejects instructions with >2 sync waits).
    _, totw = _blob_layout(S)
    blob_d = nc.declare_dram_parameter("blob", [128, totw // 2], FP32, isOutput=False)
    out_d = nc.declare_dram_parameter("out", [S, 40, HID], FP32, isOutput=True)

    with tile.TileContext(nc) as tc:
        build_body(nc, tc, S, blob_d, out_d)
    nc.compile()
    return nc


def build_body(nc, tc, S, blob_d, out_d):
    import contextlib

    ctx = contextlib.ExitStack()
    with ctx:
        const = ctx.enter_context(tc.tile_pool(name="const", bufs=1))
        psum = ctx.enter_context(tc.tile_pool(name="psum", bufs=1, space="PSUM"))
        hT_pool = ctx.enter_context(tc.tile_pool(name="hT", bufs=3))
        h_pool = ctx.enter_context(tc.tile_pool(name="h", bufs=3))
        r1_pool = ctx.enter_context(tc.tile_pool(name="r1", bufs=3))
        g_pool = ctx.enter_context(tc.tile_pool(name="g", bufs=2))

        # ---- one blob DMA; bf16 views via bitcast ----
        off, totw = _blob_layout(S)
        blob = const.tile([128, totw // 2], FP32, tag="blob", name="blob")
        nc.gpsimd.dma_start(out=blob[:], in_=blob_d[:])
        b16 = blob[:].bitcast(BF16)

        def seg(name, width):
            o = off[name]
            return b16[:, o:o + width]

        xT = [seg(f"xT{k}", S * BC) for k in range(4)]
        W, U, BRZ, BXN, BHN, OH = {}, {}, {}, {}, {}, {}
        ob = off["bias"]
        for ci, (cname, _, kx) in enumerate(ALL_CELLS):
            W[cname] = [seg(f"w_{cname}{k}", G3) for k in range(kx)]
            U[cname] = [seg(f"u_{cname}{k}", G3) for k in range(4)]
            # bias rows live on partitions 0:4; select row ci with a one-hot
            # stationary so every matmul operand stays at base partition 0
            BRZ[cname] = b16[0:4, ob:ob + 1024]
            BXN[cname] = b16[0:4, ob + 1024:ob + 1536]
            BHN[cname] = b16[0:4, ob + 1536:ob + 2048]
            oo = off["onehot"]
            OH[cname] = b16[0:4, oo + ci * BC: oo + (ci + 1) * BC]

        ident = const.tile([128, 128], FP32, tag="ident")
        make_identity(nc, ident[:])

        # ---- initial state (zeros) ----
        hT_prev = [const.tile([128, 512], BF16, tag=f"hT0_{l}", name=f"hT0_{l}") for l in range(2)]
        h_prev = [const.tile([128, 512], FP32, tag=f"h0_{l}", name=f"h0_{l}") for l in range(2)]
        for l in range(2):
            nc.any.memset(hT_prev[l][:], 0.0)
            nc.any.memset(h_prev[l][:], 0.0)

        # ---- the recurrence ----
        for t in range(S):
            h0T_new = r1_new = None
            for layer, cells in ((0, L0_CELLS), (1, L1_CELLS)):
                # stationary x-part K-chunk lhsTs per layer
                if layer == 0:
                    xchunks = [xT[k][:, t * BC:(t + 1) * BC] for k in range(4)]
                else:
                    xchunks = [h0T_new[:, k * 128: k * 128 + BC] for k in range(4)] + \
                              [r1_new[:, k * BC:(k + 1) * BC] for k in range(4)]

                gh = psum.tile([128, 2048], FP32, tag="gh", bufs=1)
                pr = gh[:, 0:512]
                pz = gh[:, 512:1024]
                pxn = gh[:, 1024:1536]
                phn = gh[:, 1536:2048]

                # matmuls: interleave cells for PE column-group concurrency
                kx = cells[0][2]
                hchunks = {c: [hT_prev[layer][:, k * 128 + b: k * 128 + b + BC] for k in range(4)]
                           for (c, b, _) in cells}
                for tgt, col, use_x, use_h, bias in (
                    (pr, 0, True, True, "rz0"),
                    (pz, 512, True, True, "rz1"),
                    (pxn, 1024, True, False, "xn"),
                    (phn, 1024, False, True, "hn"),
                ):
                    nk = (kx if use_x else 0) + (4 if use_h else 0)
                    for k in range(nk):
                        for (cname, b, _) in cells:
                            if use_x and k < kx:
                                lhsT = xchunks[k]
                                rhs = W[cname][k][:, col:col + 512]
                            else:
                                kh = k - (kx if use_x else 0)
                                lhsT = hchunks[cname][kh]
                                rhs = U[cname][kh][:, col:col + 512]
                            nc.tensor.matmul(
                                out=tgt[b:b + BC, :], lhsT=lhsT, rhs=rhs,
                                start=(k == 0), stop=False)
                    # bias via ones-row (K=1)
                    for (cname, b, _) in cells:
                        if bias == "rz0":
                            brhs = BRZ[cname][:, 0:512]
                        elif bias == "rz1":
                            brhs = BRZ[cname][:, 512:1024]
                        elif bias == "xn":
                            brhs = BXN[cname]
                        else:
                            brhs = BHN[cname]
                        nc.tensor.matmul(out=tgt[b:b + BC, :], lhsT=OH[cname],
                                         rhs=brhs, start=False, stop=True)

                # ---- gates (partitions 0:40 cover both cells) ----
                P40 = 40
                r_sb = g_pool.tile([128, 512], BF16, tag="r")
                z_sb = g_pool.tile([128, 512], BF16, tag="z")
                u_sb = g_pool.tile([128, 512], FP32, tag="u")
                v_sb = g_pool.tile([128, 512], FP32, tag="v")
                n_sb = g_pool.tile([128, 512], FP32, tag="n")
                d_sb = g_pool.tile([128, 512], FP32, tag="d")
                e_sb = g_pool.tile([128, 512], FP32, tag="e")
                h2 = h_pool.tile([128, 512], FP32, tag="h2")

                ACT = mybir.ActivationFunctionType
                OP = mybir.AluOpType
                nc.scalar.activation(r_sb[:P40, :], pr[:P40, :], ACT.Sigmoid)
                nc.scalar.activation(z_sb[:P40, :], pz[:P40, :], ACT.Sigmoid)
                nc.vector.tensor_tensor(out=u_sb[:P40, :], in0=r_sb[:P40, :], in1=phn[:P40, :], op=OP.mult)
                nc.vector.tensor_tensor(out=v_sb[:P40, :], in0=u_sb[:P40, :], in1=pxn[:P40, :], op=OP.add)
                nc.scalar.activation(n_sb[:P40, :], v_sb[:P40, :], ACT.Tanh)
                nc.vector.tensor_tensor(out=d_sb[:P40, :], in0=h_prev[layer][:P40, :], in1=n_sb[:P40, :], op=OP.subtract)
                nc.vector.tensor_tensor(out=e_sb[:P40, :], in0=z_sb[:P40, :], in1=d_sb[:P40, :], op=OP.mult)
                nc.vector.tensor_tensor(out=h2[:P40, :], in0=n_sb[:P40, :], in1=e_sb[:P40, :], op=OP.add)

                # ---- transpose h2 -> hT (and reversed copy for layer 0) ----
                ptr = psum.tile([128, 512], FP32, tag="ptr", bufs=2)
                for k in range(4):
                    nc.tensor.transpose(
                        out=ptr[:, k * 128:(k + 1) * 128],
                        in_=h2[:, k * 128:(k + 1) * 128],
                        identity=ident[:])
                hT_new = hT_pool.tile([128, 512], BF16, tag=f"hTn")
                nc.vector.tensor_copy(out=hT_new[:], in_=ptr[:])

                if layer == 0:
                    r1_new = r1_pool.tile([128, 4 * BC], BF16, tag="r1")
                    src = ptr.rearrange("p (k c) -> p k c", k=4)[:, :, 39:31:-1]
                    nc.vector.tensor_copy(
                        out=r1_new.rearrange("p (k c) -> p k c", k=4)[:, :, :],
                        in_=src)
                    h0T_new = hT_new
                else:
                    nc.gpsimd.dma_start(out=out_d[t, :, :], in_=h2[0:40, :])

                hT_prev[layer] = hT_new
                h_prev[layer] = h2


# ---------------------------------------------------------------------------
# host side
# ---------------------------------------------------------------------------

_CACHE = {}


def _groups():
    return [list(range(4 * d, 4 * d + 4)) + [63 - (4 * d + 3), 63 - (4 * d + 2),
            63 - (4 * d + 1), 63 - 4 * d] for d in range(NCORES)]


def _bf16_u16(a):
    """Round fp32 array to bf16 bit patterns (uint16, round-nearest-even)."""
    a = np.ascontiguousarray(a, np.float32)
    u = a.view(np.uint32)
    return ((u + 0x7FFF + ((u >> 16) & 1)) >> 16).astype(np.uint16)


def _pack_words(u16):
    """Pack a (..., 2N) uint16 array into (..., N) float32 words."""
    ev = u16[..., 0::2].astype(np.uint32)
    od = u16[..., 1::2].astype(np.uint32)
    return (ev | (od << 16)).view(np.float32)


def _blob_shared_parts(inputs, S):
    """uint16 blob segments that are identical across cores."""
    off, totw = _blob_layout(S)
    parts = []
    for cname in ("f0", "b0", "f1", "b1"):
        wih = np.asarray(inputs[f"wih_{cname}"], np.float32)   # (1536, in)
        whh = np.asarray(inputs[f"whh_{cname}"], np.float32)   # (1536, 512)
        kin = wih.shape[1] // 128
        parts.append((off[f"w_{cname}0"], _bf16_u16(
            wih.T.reshape(kin, 128, G3)).transpose(1, 0, 2).reshape(128, kin * G3)))
        parts.append((off[f"u_{cname}0"], _bf16_u16(
            whh.T.reshape(4, 128, G3)).transpose(1, 0, 2).reshape(128, 4 * G3)))
    brows = np.zeros((128, 2048), np.uint16)
    for ci, cname in enumerate(("f0", "b0", "f1", "b1")):
        bih = np.asarray(inputs[f"bih_{cname}"], np.float32)
        bhh = np.asarray(inputs[f"bhh_{cname}"], np.float32)
        b = np.concatenate([(bih + bhh)[:1024], bih[1024:], bhh[1024:]])
        brows[ci, :] = _bf16_u16(b)
    parts.append((off["bias"], brows))
    ohrows = np.zeros((128, 4 * 8), np.uint16)
    one = _bf16_u16(np.ones(1, np.float32))[0]
    for ci in range(4):
        ohrows[ci, ci * 8:(ci + 1) * 8] = one
    parts.append((off["onehot"], ohrows))
    return parts, off, totw


class _Runner:
    """Caches the traced+compiled SPMD executable so repeat calls skip the
    (expensive) jax retrace and BIR re-serialization."""

    def __init__(self, S):
        import jax
        from jax.sharding import Mesh, PartitionSpec
        from jax.experimental.shard_map import shard_map
        from concourse import bass2jax
        from concourse.bass2jax import _bass_exec_p, partition_id_tensor

        bass2jax.install_neuronx_cc_hook()
        self.S = S
        nc = build_core_program(S)
        self.nc = nc
        partition_name = nc.partition_id_tensor.name if nc.partition_id_tensor else None
        in_names, out_names, out_avals, zero_outs = [], [], [], []
        for alloc in nc.m.functions[0].allocations:
            if not isinstance(alloc, mybir.MemoryLocationSet):
                continue
            name = alloc.memorylocations[0].name
            if alloc.kind == "ExternalInput":
                if name != partition_name:
                    in_names.append(name)
            elif alloc.kind == "ExternalOutput":
                shape = tuple(alloc.tensor_shape)
                dtype = mybir.dt.np(alloc.dtype)
                out_names.append(name)
                out_avals.append(jax.core.ShapedArray(shape, dtype))
                zero_outs.append(np.zeros(shape, dtype))
        n_params = len(in_names)
        self.in_names = list(in_names)
        self.out_names = out_names
        self.out_shapes = [tuple(a.shape) for a in out_avals]
        self.zero_outs = zero_outs
        all_in = in_names + out_names + ([partition_name] if partition_name else [])

        def _body(*args):
            operands = list(args)
            if partition_name is not None:
                operands.append(partition_id_tensor())
            return tuple(_bass_exec_p.bind(
                *operands,
                out_avals=tuple(out_avals),
                in_names=tuple(all_in),
                out_names=tuple(out_names),
                lowering_input_output_aliases=(),
                sim_require_finite=True,
                sim_require_nnan=True,
                nc=nc,
            ))

        devices = jax.devices()[:NCORES]
        mesh = Mesh(np.asarray(devices), ("core",))
        in_specs = (PartitionSpec("core"),) * (n_params + len(out_names))
        out_specs = (PartitionSpec("core"),) * len(out_names)
        self.fn = jax.jit(
            shard_map(_body, mesh=mesh, in_specs=in_specs,
                      out_specs=out_specs, check_rep=False),
            keep_unused=True)
        self.jax = jax

    def run(self, in_maps):
        concat_in = [
            np.concatenate([np.asarray(m[nm]) for m in in_maps], axis=0)
            for nm in self.in_names]
        concat_zero = [np.zeros((NCORES * z.shape[0], *z.shape[1:]), z.dtype)
                       for z in self.zero_outs]
        outs = self.fn(*concat_in, *concat_zero)
        return [
            {nm: np.asarray(outs[i]).reshape(NCORES, *self.out_shapes[i])[c]
             for i, nm in enumerate(self.out_names)}
            for c in range(NCORES)]

    def run_timed(self, in_maps, iters=5):
        """Stage inputs on device, then time bare executions."""
        import time
        concat_in = [
            self.jax.device_put(np.concatenate(
                [np.asarray(m[nm]) for m in in_maps], axis=0))
            for nm in self.in_names]
        concat_zero = [np.zeros((NCORES * z.shape[0], *z.shape[1:]), z.dtype)
                       for z in self.zero_outs]
        o = self.fn(*concat_in, *concat_zero)
        self.jax.block_until_ready(o)
        best = float("inf")
        for _ in range(iters):
            t0 = time.perf_counter()
            o = self.fn(*concat_in, *concat_zero)
            self.jax.block_until_ready(o)
            best = min(best, time.perf_counter() - t0)
        return best


def _in_maps(inputs):
    S = inputs["x"].shape[0]
    x = np.asarray(inputs["x"], np.float32)
    parts, off, totw = _blob_shared_parts(inputs, S)
    groups = _groups()
    shared_blob = np.zeros((128, totw), np.uint16)
    for o, seg in parts:
        shared_blob[:, o:o + seg.shape[1]] = seg
    in_maps = []
    for d in range(NCORES):
        xl = x[:, groups[d], :]                       # (S, 8, 512)
        blob = shared_blob.copy()
        xseg = _bf16_u16(xl.transpose(2, 0, 1).reshape(4, 128, S * BC))
        blob[:, off["xT0"]:off["xT0"] + 4 * S * BC] = \
            xseg.transpose(1, 0, 2).reshape(128, 4 * S * BC)
        in_maps.append({"blob": _pack_words(blob)})
    return in_maps


def _assemble(outs, S):
    groups = _groups()
    out = np.zeros((S, BATCH, 2 * HID), np.float32)
    for d in range(NCORES):
        raw = outs[d]["out"]                          # (S, 40, 512)
        G = groups[d]
        for b in range(BC):
            out[:, G[b], 0:HID] = raw[:, b, :]
            out[:, G[b], HID:] = raw[:, 32 + 7 - b, :]
    return out


def kernel(**inputs):
    S = inputs["x"].shape[0]
    if S not in _CACHE:
        _CACHE[S] = _Runner(S)
    runner = _CACHE[S]
    outs = runner.run(_in_maps(inputs))
    return _assemble(outs, S)


if __name__ == "__main__":
    rng = np.random.default_rng(0)
    S = 16
    inputs = {"x": rng.standard_normal((S, 64, 512), dtype=np.float32)}
    s = 1.0 / np.sqrt(HID)
    u = lambda *shp: rng.uniform(-s, s, shp).astype(np.float32)
    for c, idim in (("f0", 512), ("b0", 512), ("f1", 1024), ("b1", 1024)):
        inputs[f"wih_{c}"] = u(G3, idim)
        inputs[f"whh_{c}"] = u(G3, HID)
        inputs[f"bih_{c}"] = u(G3)
        inputs[f"bhh_{c}"] = u(G3)
    out = kernel(**inputs)
    print("kernel ran, out", out.shape, float(np.abs(out).mean()))


# revision 29
# speedup vs baseline: 27.1103x; 27.1103x over previous
"""Trainium2 Bass kernel for a 2-layer "BiGRU" (batch-flipped, per reference).

Structure exploited:
  * The reference's "backward" direction flips the BATCH dim, not time. In
    flipped coordinates (track hb_hat[b] := hb[B-1-b]) every GRU cell
    consumes the UNFLIPPED input stream; flips appear only when building
    layer-1's input concat and in the final output (host side).
  * Batch 64 is sharded over 8 cores in flip-closed groups of 8, so the
    flip is a local batch reversal and cores are fully independent.
  * The input-side matmuls (x@wihT + biases) are bulk-precomputed at full
    PE utilization: layer-0's in a prepass over all steps; layer-1's in
    16-step chunks as the layer-0 wavefront produces its inputs. The
    recurrent loop streams only whh through the PE (h^T stationary), with
    all four GRU cells in different PE column groups (output partition
    bases 0/32/64/96) so their weight streams run concurrently. Layer 1
    runs LAG steps behind layer 0 in the same iteration, sharing one PSUM
    tile, so the elementwise gate math covers all four cells per op.

Self-contained: hardcodes all shapes from the problem spec.
"""

import numpy as np

from concourse import bass, bacc, tile
from concourse.bass import mybir
from concourse.masks import make_identity

SEQ, BATCH, IN, HID = 512, 64, 512, 512
G3 = 3 * HID  # 1536
BC = 8        # local batch per core
NCORES = 8
CH = 16       # wavefront chunk (steps) for layer-1 input bulk matmuls
LAG = 24      # layer-1 lag behind layer-0 (> CH, multiple of W4)
W4 = 4        # gi DMA window (steps)
FP32 = mybir.dt.float32
BF16 = mybir.dt.bfloat16

# cell name, PSUM base partition, input K-chunks of 128
CELLS = [("f0", 0, 4), ("b0", 32, 4), ("f1", 64, 8), ("b1", 96, 8)]


def _blob_layout():
    """Free-dim offsets (in bf16 elements) inside the single load blob."""
    off = {}
    cur = 0
    for cname, _, kx in CELLS:
        for k in range(kx):
            off[f"w_{cname}{k}"] = cur
            cur += G3
        for k in range(4):
            off[f"u_{cname}{k}"] = cur
            cur += G3
    off["bias"] = cur      # rows 0:4 = cells; per cell: [bulk_bias 1536 | bhn 512]
    cur += 2048
    off["ohrow"] = cur     # rows 0:4; cell c: cols c*128..+128 = 1.0 in row c
    cur += 4 * 128
    off["i8"] = cur        # identity 8x8 replicated at partition bands 0/32/64/96
    cur += 8
    return off, cur


def build_core_program(S):
    assert S % CH == 0 and LAG % W4 == 0 and CH % W4 == 0
    nc = bacc.Bacc(None, target_bir_lowering=False)

    off, totw = _blob_layout()
    blob_d = nc.declare_dram_parameter("blob", [128, totw // 2], FP32, isOutput=False)
    xTp_d = nc.declare_dram_parameter("xTp", [128, 4, S * BC // 2], FP32, isOutput=False)
    out_d = nc.declare_dram_parameter("out", [S, 40, HID], BF16, isOutput=True)

    with tile.TileContext(nc) as tc:
        build_body(nc, tc, S, blob_d, xTp_d, out_d, off, totw)
    nc.compile()
    return nc


def build_body(nc, tc, S, blob_d, xTp_d, out_d, off, totw):
    import contextlib

    ACT = mybir.ActivationFunctionType
    OP = mybir.AluOpType
    NCHUNK = S // CH

    ctx = contextlib.ExitStack()
    with ctx:
        const = ctx.enter_context(tc.tile_pool(name="const", bufs=1))
        ghp = ctx.enter_context(tc.tile_pool(name="ghp", bufs=1, space="PSUM"))
        scr = ctx.enter_context(tc.tile_pool(name="scr", bufs=1, space="PSUM"))
        dram = ctx.enter_context(tc.tile_pool(name="dram", bufs=1, space="DRAM"))
        xr_pool = ctx.enter_context(tc.tile_pool(name="xr", bufs=2))
        ev_pool = ctx.enter_context(tc.tile_pool(name="ev", bufs=3))
        ring_pool = ctx.enter_context(tc.tile_pool(name="ring", bufs=3))
        buf_pool = ctx.enter_context(tc.tile_pool(name="buf", bufs=4))
        hT1_pool = ctx.enter_context(tc.tile_pool(name="hT1", bufs=3))
        g_pool = ctx.enter_context(tc.tile_pool(name="g", bufs=3))

        # ---- load blob (single DMA), bf16 views via bitcast ----
        blob = const.tile([128, totw // 2], FP32, tag="blob", name="blob")
        nc.gpsimd.dma_start(out=blob[:], in_=blob_d[:])
        b16 = blob[:].bitcast(BF16)

        W, U, BULKB, BHN, OHR, I8 = {}, {}, {}, {}, {}, {}
        ob, oh, oi = off["bias"], off["ohrow"], off["i8"]
        for ci, (cname, base, kx) in enumerate(CELLS):
            W[cname] = [b16[:, off[f"w_{cname}{k}"]:off[f"w_{cname}{k}"] + G3]
                        for k in range(kx)]
            U[cname] = [b16[:, off[f"u_{cname}{k}"]:off[f"u_{cname}{k}"] + G3]
                        for k in range(4)]
            BULKB[cname] = b16[0:4, ob:ob + 1536]      # row ci is live
            BHN[cname] = b16[0:4, ob + 1536:ob + 2048]
            OHR[cname] = b16[0:4, oh + ci * 128:oh + (ci + 1) * 128]
            I8[cname] = b16[base:base + 8, oi:oi + 8]

        zero16 = const.tile([128, 512], BF16, tag="zero16", name="zero16")
        nc.any.memset(zero16[:], 0.0)
        ident = const.tile([128, 128], BF16, tag="ident")
        make_identity(nc, ident[:])

        # per-cell bulk bias row selector uses rows 0:4 of the bias segment;
        # BULKB/BHN slices are shared APs — the one-hot picks the row.

        # ---- internal DRAM for bulk gi results ----
        gi0_dram = {c: dram.tile([S * BC, G3], BF16, tag=f"gi0_{c}", name=f"gi0_{c}")
                    for c in ("f0", "b0")}
        gi1_dram = {c: [dram.tile([CH * BC, G3], BF16, tag=f"gi1_{c}", bufs=4,
                                  name=f"gi1_{c}_{cc}")
                        for cc in range(NCHUNK)]
                    for c in ("f1", "b1")}

        def bulk_gi(cell, lhs_chunks, out_rows_ap):
            """One 128-row tile of gi = x @ wihT + bias -> DRAM (bf16)."""
            kx = len(lhs_chunks)
            for n in range(3):
                ps = scr.tile([128, 512], FP32, tag="scr", bufs=2)
                for k in range(kx):
                    nc.tensor.matmul(out=ps[:], lhsT=lhs_chunks[k],
                                     rhs=W[cell][k][:, n * 512:(n + 1) * 512],
                                     start=(k == 0), stop=False)
                nc.tensor.matmul(out=ps[:], lhsT=OHR[cell],
                                 rhs=BULKB[cell][:, n * 512:(n + 1) * 512],
                                 start=False, stop=True)
                ev = ev_pool.tile([128, 512], BF16, tag="ev")
                nc.vector.tensor_copy(out=ev[:], in_=ps[:])
                nc.gpsimd.dma_start(out=out_rows_ap[:, n * 512:(n + 1) * 512],
                                    in_=ev[:])

        # ---- prepass: gi0 for all steps ----
        for rt in range(S * BC // 128):
            xrt = xr_pool.tile([128, 4, 64], FP32, tag="xr")
            nc.gpsimd.dma_start(out=xrt[:], in_=xTp_d[:, :, rt * 64:(rt + 1) * 64])
            x16 = xrt[:].bitcast(BF16)   # (128, 4, 128)
            for cell in ("f0", "b0"):
                bulk_gi(cell, [x16[:, k, :] for k in range(4)],
                        gi0_dram[cell][rt * 128:(rt + 1) * 128, :])

        # ---- wavefront loop: L0 at step i, L1 at step i-LAG ----
        bufA, bufB, bufC = {}, {}, {}   # chunk -> (128, 4, CH*BC) tiles
        hT1_tiles = {}                  # t1 -> (128, 4, 64) tile
        h_prev = zero16
        ring = None
        for i in range(S + LAG):
            t0, t1 = i, i - LAG         # layer-0 / layer-1 step indices
            cc0, cc1 = t0 // CH, t1 // CH

            if i % W4 == 0:
                ring = ring_pool.tile([128, W4 * G3], BF16, tag="ring")
                rv = ring.rearrange("p (s g) -> p s g", s=W4)
                if t0 < S:
                    for cell, base in (("f0", 0), ("b0", 32)):
                        src = gi0_dram[cell][:].rearrange(
                            "(s b) g -> b s g", b=BC)[:, t0:t0 + W4, :]
                        nc.gpsimd.dma_start(out=rv[base:base + BC], in_=src)
                if t1 >= 0:
                    for cell, base in (("f1", 64), ("b1", 96)):
                        src = gi1_dram[cell][cc1][:].rearrange(
                            "(s b) g -> b s g", b=BC)[:, t1 % CH:t1 % CH + W4, :]
                        nc.gpsimd.dma_start(out=rv[base:base + BC], in_=src)
            wi = i % W4

            if t0 < S and t0 % CH == 0:
                bufA[cc0] = buf_pool.tile([128, 4, CH * BC], BF16, tag="bufA",
                                          name=f"bufA{cc0}")
                bufB[cc0] = buf_pool.tile([128, 4, CH * BC], BF16, tag="bufB",
                                          name=f"bufB{cc0}")
                bufC[cc0] = buf_pool.tile([128, 4, CH * BC], BF16, tag="bufC",
                                          name=f"bufC{cc0}")

            # ---------- matmuls into one shared PSUM tile ----------
            gh = ghp.tile([128, G3], FP32, tag="gh", bufs=2)

            def hch(cell, k):
                if cell == "f0":
                    cb, sl = bufA[(t0 - 1) // CH], ((t0 - 1) % CH) * BC
                    return cb[:, k, sl:sl + BC]
                if cell == "b0":
                    cb, sl = bufB[(t0 - 1) // CH], ((t0 - 1) % CH) * BC
                    return cb[:, k, sl:sl + BC]
                hb = 0 if cell == "f1" else 32
                return hT1_tiles[t1 - 1][:, k, hb:hb + BC]

            def xch1(k):   # layer-1 input chunks: [hf0T | flip(hb0T)] at step t1
                cb = bufA[t1 // CH] if k < 4 else bufC[t1 // CH]
                sl = (t1 % CH) * BC
                return cb[:, k % 4, sl:sl + BC]

            live = []
            if t0 < S:
                live += [("f0", 0), ("b0", 32)]
            if t1 >= 0:
                live += [("f1", 64), ("b1", 96)]

            def step_of(cell):
                return t0 if cell in ("f0", "b0") else t1

            # r and z slices: gi inject (start) + 4 whh chunks
            for n, col in ((0, 0), (1, 512)):
                for cell, base in live:
                    nc.tensor.matmul(
                        out=gh[base:base + BC, col:col + 512], lhsT=I8[cell],
                        rhs=ring[base:base + BC, wi * G3 + col:wi * G3 + col + 512],
                        start=True, stop=(step_of(cell) == 0),
                        tile_position=(base, base))
                for k in range(4):
                    for cell, base in live:
                        if step_of(cell) == 0:
                            continue
                        nc.tensor.matmul(
                            out=gh[base:base + BC, col:col + 512],
                            lhsT=hch(cell, k),
                            rhs=U[cell][k][:, col:col + 512],
                            start=False, stop=(k == 3),
                            tile_position=(0, base))
            # hn slice: bhh_n inject (start) + 4 whh chunks
            for cell, base in live:
                nc.tensor.matmul(
                    out=gh[base:base + BC, 1024:1536], lhsT=OHR[cell][:, 0:BC],
                    rhs=BHN[cell], start=True, stop=(step_of(cell) == 0),
                    tile_position=(0, base))
            for k in range(4):
                for cell, base in live:
                    if step_of(cell) == 0:
                        continue
                    nc.tensor.matmul(
                        out=gh[base:base + BC, 1024:1536],
                        lhsT=hch(cell, k),
                        rhs=U[cell][k][:, 1024:1536],
                        start=False, stop=(k == 3),
                        tile_position=(0, base))

            # ---------- gates: one op covers all four cells ----------
            rz = g_pool.tile([128, 1024], BF16, tag="rz")
            u16 = g_pool.tile([128, 512], BF16, tag="u16")
            v16 = g_pool.tile([128, 512], BF16, tag="v16")
            n16 = g_pool.tile([128, 512], BF16, tag="n16")
            d16 = g_pool.tile([128, 512], BF16, tag="d16")
            e16 = g_pool.tile([128, 512], BF16, tag="e16")
            h2 = g_pool.tile([128, 512], BF16, tag="h2")

            nc.scalar.activation(rz[:], gh[:, 0:1024], ACT.Sigmoid)
            nc.vector.tensor_tensor(out=u16[:], in0=rz[:, 0:512],
                                    in1=gh[:, 1024:1536], op=OP.mult)
            nc.vector.tensor_tensor(
                out=v16[:], in0=u16[:],
                in1=ring[:, wi * G3 + 1024:wi * G3 + 1536], op=OP.add)
            nc.scalar.activation(n16[:], v16[:], ACT.Tanh)
            nc.vector.tensor_tensor(out=d16[:], in0=h_prev[:], in1=n16[:],
                                    op=OP.subtract)
            nc.vector.tensor_tensor(out=e16[:], in0=rz[:, 512:1024], in1=d16[:],
                                    op=OP.mult)
            nc.vector.tensor_tensor(out=h2[:], in0=n16[:], in1=e16[:], op=OP.add)
            if i == LAG - 1:
                nc.any.memset(h2[64:128, :], 0.0)
            h_prev = h2

            # ---------- transpose h2 -> hT layouts ----------
            ptr = scr.tile([128, 4, 128], BF16, tag="scr", bufs=2)
            for k in range(4):
                nc.tensor.transpose(out=ptr[:, k, :], in_=h2[:, k * 128:(k + 1) * 128],
                                    identity=ident[:])
            if t0 < S:
                sl = (t0 % CH) * BC
                nc.vector.tensor_copy(out=bufA[cc0][:, :, sl:sl + BC],
                                      in_=ptr[:, :, 0:BC])
                nc.vector.tensor_copy(out=bufB[cc0][:, :, sl:sl + BC],
                                      in_=ptr[:, :, 32:32 + BC])
                nc.vector.tensor_copy(out=bufC[cc0][:, :, sl:sl + BC],
                                      in_=ptr[:, :, 39:31:-1])
            hT1 = hT1_pool.tile([128, 4, 64], BF16, tag="hT1")
            nc.vector.tensor_copy(out=hT1[:, :, 0:40], in_=ptr[:, :, 64:104])
            hT1_tiles[t1] = hT1

            # ---------- layer-1 output (bf16, direct from h2) ----------
            if t1 >= 0:
                nc.gpsimd.dma_start(out=out_d[t1, :, :], in_=h2[64:104, :])

            # ---------- bulk gi1 for the chunk layer-0 just finished ----------
            if t0 < S and t0 % CH == CH - 1:
                for cell in ("f1", "b1"):
                    lhs = [bufA[cc0][:, k, :] for k in range(4)] + \
                          [bufC[cc0][:, k, :] for k in range(4)]
                    bulk_gi(cell, lhs, gi1_dram[cell][cc0][:, :])


# ---------------------------------------------------------------------------
# host side
# ---------------------------------------------------------------------------

_CACHE = {}


def _groups():
    return [list(range(4 * d, 4 * d + 4)) + [63 - (4 * d + 3), 63 - (4 * d + 2),
            63 - (4 * d + 1), 63 - 4 * d] for d in range(NCORES)]


def _bf16_u16(a):
    a = np.ascontiguousarray(a, np.float32)
    u = a.view(np.uint32)
    return ((u + 0x7FFF + ((u >> 16) & 1)) >> 16).astype(np.uint16)


def _pack_words(u16):
    ev = u16[..., 0::2].astype(np.uint32)
    od = u16[..., 1::2].astype(np.uint32)
    return (ev | (od << 16)).view(np.float32)


def _blob_host(inputs):
    off, totw = _blob_layout()
    blob = np.zeros((128, totw), np.uint16)
    for ci, (cname, base, kx) in enumerate(CELLS):
        wih = np.asarray(inputs[f"wih_{cname}"], np.float32)   # (1536, in)
        whh = np.asarray(inputs[f"whh_{cname}"], np.float32)   # (1536, 512)
        bih = np.asarray(inputs[f"bih_{cname}"], np.float32)
        bhh = np.asarray(inputs[f"bhh_{cname}"], np.float32)
        wt = _bf16_u16(wih.T.reshape(kx, 128, G3))
        ut = _bf16_u16(whh.T.reshape(4, 128, G3))
        for k in range(kx):
            o = off[f"w_{cname}{k}"]
            blob[:, o:o + G3] = wt[k]
        for k in range(4):
            o = off[f"u_{cname}{k}"]
            blob[:, o:o + G3] = ut[k]
        bulkb = np.concatenate([(bih + bhh)[:1024], bih[1024:]])
        blob[ci, off["bias"]:off["bias"] + 1536] = _bf16_u16(bulkb)
        blob[ci, off["bias"] + 1536:off["bias"] + 2048] = _bf16_u16(bhh[1024:])
        blob[ci, off["ohrow"] + ci * 128:off["ohrow"] + (ci + 1) * 128] = \
            _bf16_u16(np.ones(128, np.float32))
    one = _bf16_u16(np.ones(1, np.float32))[0]
    for base in (0, 32, 64, 96):
        for j in range(8):
            blob[base + j, off["i8"] + j] = one
    return _pack_words(blob)


def _in_maps(inputs):
    S = inputs["x"].shape[0]
    x = np.asarray(inputs["x"], np.float32)
    groups = _groups()
    blob = _blob_host(inputs)
    in_maps = []
    for d in range(NCORES):
        xl = x[:, groups[d], :]                       # (S, 8, 512)
        # xTp layout: (128 part, 4 k, S*BC) -> words
        xT = _bf16_u16(xl.transpose(2, 0, 1).reshape(4, 128, S * BC))
        xT = np.ascontiguousarray(xT.transpose(1, 0, 2))   # (128, 4, S*BC)
        in_maps.append({"blob": blob, "xTp": _pack_words(xT)})
    return in_maps


def _assemble(outs, S):
    groups = _groups()
    out = np.zeros((S, BATCH, 2 * HID), np.float32)
    for d in range(NCORES):
        raw = np.asarray(outs[d]["out"], np.float32)  # (S, 40, 512)
        G = groups[d]
        for b in range(BC):
            out[:, G[b], 0:HID] = raw[:, b, :]
            out[:, G[b], HID:] = raw[:, 32 + 7 - b, :]
    return out


class _Runner:
    """Caches the traced+compiled SPMD executable so repeat calls skip the
    (expensive) jax retrace and BIR re-serialization."""

    def __init__(self, S):
        import jax
        from jax.sharding import Mesh, PartitionSpec
        from jax.experimental.shard_map import shard_map
        from concourse import bass2jax
        from concourse.bass2jax import _bass_exec_p, partition_id_tensor

        bass2jax.install_neuronx_cc_hook()
        self.S = S
        nc = build_core_program(S)
        self.nc = nc
        partition_name = nc.partition_id_tensor.name if nc.partition_id_tensor else None
        in_names, out_names, out_avals, zero_outs = [], [], [], []
        for alloc in nc.m.functions[0].allocations:
            if not isinstance(alloc, mybir.MemoryLocationSet):
                continue
            name = alloc.memorylocations[0].name
            if alloc.kind == "ExternalInput":
                if name != partition_name:
                    in_names.append(name)
            elif alloc.kind == "ExternalOutput":
                shape = tuple(alloc.tensor_shape)
                dtype = mybir.dt.np(alloc.dtype)
                out_names.append(name)
                out_avals.append(jax.core.ShapedArray(shape, dtype))
                zero_outs.append(np.zeros(shape, dtype))
        n_params = len(in_names)
        self.in_names = list(in_names)
        self.out_names = out_names
        self.out_shapes = [tuple(a.shape) for a in out_avals]
        self.zero_outs = zero_outs
        all_in = in_names + out_names + ([partition_name] if partition_name else [])

        def _body(*args):
            operands = list(args)
            if partition_name is not None:
                operands.append(partition_id_tensor())
            return tuple(_bass_exec_p.bind(
                *operands,
                out_avals=tuple(out_avals),
                in_names=tuple(all_in),
                out_names=tuple(out_names),
                lowering_input_output_aliases=(),
                sim_require_finite=True,
                sim_require_nnan=True,
                nc=nc,
            ))

        devices = jax.devices()[:NCORES]
        mesh = Mesh(np.asarray(devices), ("core",))
        in_specs = (PartitionSpec("core"),) * (n_params + len(out_names))
        out_specs = (PartitionSpec("core"),) * len(out_names)
        self.fn = jax.jit(
            shard_map(_body, mesh=mesh, in_specs=in_specs,
                      out_specs=out_specs, check_rep=False),
            keep_unused=True)
        self.jax = jax

    def run(self, in_maps):
        concat_in = [
            np.concatenate([np.asarray(m[nm]) for m in in_maps], axis=0)
            for nm in self.in_names]
        concat_zero = [np.zeros((NCORES * z.shape[0], *z.shape[1:]), z.dtype)
                       for z in self.zero_outs]
        outs = self.fn(*concat_in, *concat_zero)
        return [
            {nm: np.asarray(outs[i]).reshape(NCORES, *self.out_shapes[i])[c]
             for i, nm in enumerate(self.out_names)}
            for c in range(NCORES)]

    def run_timed(self, in_maps, iters=5):
        """Stage inputs (and the pre-zeroed output buffers — every output
        element is written, so reuse is safe) on device; time executions."""
        import time
        concat_in = [
            self.jax.device_put(np.concatenate(
                [np.asarray(m[nm]) for m in in_maps], axis=0))
            for nm in self.in_names]
        concat_zero = [
            self.jax.device_put(
                np.zeros((NCORES * z.shape[0], *z.shape[1:]), z.dtype))
            for z in self.zero_outs]
        o = self.fn(*concat_in, *concat_zero)
        self.jax.block_until_ready(o)
        best = float("inf")
        for _ in range(iters):
            t0 = time.perf_counter()
            o = self.fn(*concat_in, *concat_zero)
            self.jax.block_until_ready(o)
            best = min(best, time.perf_counter() - t0)
        return best


def kernel(**inputs):
    S = inputs["x"].shape[0]
    if S not in _CACHE:
        _CACHE[S] = _Runner(S)
    runner = _CACHE[S]
    outs = runner.run(_in_maps(inputs))
    return _assemble(outs, S)


if __name__ == "__main__":
    rng = np.random.default_rng(0)
    S = 32
    inputs = {"x": rng.standard_normal((S, 64, 512), dtype=np.float32)}
    s = 1.0 / np.sqrt(HID)
    u = lambda *shp: rng.uniform(-s, s, shp).astype(np.float32)
    for c, idim in (("f0", 512), ("b0", 512), ("f1", 1024), ("b1", 1024)):
        inputs[f"wih_{c}"] = u(G3, idim)
        inputs[f"whh_{c}"] = u(G3, HID)
        inputs[f"bih_{c}"] = u(G3)
        inputs[f"bhh_{c}"] = u(G3)
    out = kernel(**inputs)
    print("kernel ran, out", out.shape, float(np.abs(out).mean()))
